# revision 25
# baseline (speedup 1.0000x reference)
"""PoPE attention kernel for Trainium2, sharded over 8 NeuronCores by heads.

Problem: B=1, S=2048, DIM=1024, H=16 heads, D=64.
  q/k/v = x @ w{q,k,v}^T ; PoPE embed (softplus magnitude x cos/sin phase);
  scores = q_emb @ k_emb^T / sqrt(D); softmax; out = attn @ v; y = out @ wo^T.

Sharding: 2 heads per core. Each core computes its heads' projections,
attention, and a partial output projection (its 128 channels of wo);
host sums the 8 partial y's (f32) - no on-chip collectives.

Layouts are "transposed" (feature-major): xT [DIM, S] so that every
matmul has its contraction on the partition axis with no on-chip
transposes. All matmuls are bf16 with f32 PSUM accumulate.
The softmax skips max-subtraction (scores/8 are bounded ~4.3), and the
rowsum comes free from a ones-column appended to v in the attn@v matmul.
Attention is pipelined in (head, half-of-keys) stages so exp buffers fit
SBUF; attn@v of the previous stage overlaps scores+exp of the current,
and the v-projection fills TensorE gaps during the first stage.
The softmax 1/rowsum uses a DMA reshape to spread the row over 128
partitions (DVE reciprocal is ~6.3ns per element per lane; [1,2048]
would cost 13us, [128,16] costs ~0.1us).
"""
import math

import numpy as np
import ml_dtypes

import concourse.bacc as bacc
import concourse.mybir as mybir
from concourse import tile
from concourse.bass_utils import run_bass_kernel_spmd

BF16 = ml_dtypes.bfloat16
S, DIM, H, D = 2048, 1024, 16, 64
NCORES = 8
HPC = H // NCORES          # heads per core = 2
ED = 2 * D                 # embedding width per head = 128
KI = DIM // 128            # contraction chunks for projections = 8
KC = S // 128              # key-token chunks = 16
QC = S // 512              # query free-dim chunks of 512 = 4
OC = DIM // 128            # output-channel chunks = 8

_compiled_nc = None


def _build_body(nc, tc, persist, ps_pool, out_pool, xt_pool, exp_pool, ext):
    dt = mybir.dt
    AF = mybir.ActivationFunctionType
    xt_ext, w_ext, tq_ext, tk_ext, wo_ext, y_ext = ext

    # ---- HAM warmup: dummy matmuls on junk data while the input DMAs run,
    # so the PE clock-gate reaches 2.4 GHz before the real matmuls start ----
    warm_sb = persist.tile([128, 512], dt.bfloat16)
    nc.gpsimd.memset(warm_sb[:], 0.0)
    warm_ps = ps_pool.tile([128, 512], dt.float32, name="warm_ps", tag="psA")
    for i in range(16):
        nc.tensor.matmul(warm_ps[:], warm_sb[:, 0:128], warm_sb[:],
                         start=(i == 0), stop=(i == 15))

    # ---- phase A: input DMAs (w and xt first so matmuls start early) ----
    w_sb = persist.tile([128, 3, KI, ED], dt.bfloat16)
    nc.sync.dma_start(w_sb[:], w_ext[:])
    xt = xt_pool.tile([128, KI, S], dt.bfloat16)
    for ki in range(KI):
        nc.sync.dma_start(xt[:, ki, :], xt_ext[:, ki, :])
    tq_sb = persist.tile([128, 2, S], dt.bfloat16)
    nc.sync.dma_start(tq_sb[:], tq_ext[:])
    tk_sb = persist.tile([128, 2, S], dt.bfloat16)
    nc.sync.dma_start(tk_sb[:], tk_ext[:])
    wo_sb = persist.tile([128, DIM], dt.bfloat16)
    nc.sync.dma_start(wo_sb[:], wo_ext[:])
    # v with a ones column appended per (head, key chunk)
    v_sb = persist.tile([128, HPC, KC, D + 1], dt.bfloat16)
    nc.gpsimd.memset(v_sb[:, 0, :, D], 1.0)
    nc.gpsimd.memset(v_sb[:, 1, :, D], 1.0)
    emb_q = [persist.tile([128, S], dt.bfloat16, name=f"embq{h}", tag=f"embq{h}")
             for h in range(HPC)]
    emb_k = [persist.tile([128, S], dt.bfloat16, name=f"embk{h}", tag=f"embk{h}")
             for h in range(HPC)]
    outT = persist.tile([128, S], dt.bfloat16)

    # ---- phase B: q/k projections (ki-outer so MMs start after the first
    # xt chunk lands), clustered softplus ----
    psm = []
    for p in range(2):  # 0=q, 1=k
        ps_t = ps_pool.tile([128, S], dt.float32, name=f"psm{p}",
                            tag=("psA", "psB")[p])
        psm.append(ps_t)
        for ki in range(KI):
            for qc in range(QC):
                nc.tensor.matmul(
                    ps_t[:, qc * 512:(qc + 1) * 512],
                    w_sb[:, p, ki, :],
                    xt[:, ki, qc * 512:(qc + 1) * 512],
                    start=(ki == 0), stop=(ki == KI - 1),
                )
    # softplus(x) = ln(1 + e^x); both Exp's feed ONE combined Ln so the
    # scheduler cannot interleave them (exp and ln live in different ACT
    # table sets; alternation costs a ~1.3us table load each time).
    tmp = xt_pool.tile([128, 2, S], dt.float32, name="sp", tag="sp")
    mag = xt_pool.tile([128, 2, S], dt.bfloat16, name="mag", tag="mag")
    qk_mag = [mag[:, 0, :], mag[:, 1, :]]
    nc.scalar.activation(tmp[:, 0, :], psm[0][:], AF.Exp)
    nc.scalar.activation(tmp[:, 1, :], psm[1][:], AF.Exp)
    nc.scalar.activation(mag[:], tmp[:], AF.Ln, bias=1.0)

    # ---- phase C: v projection (token-major directly) ----
    def v_group(g, tag):
        psv = ps_pool.tile([128, 4, 128], dt.float32, name=f"psv{g}", tag=tag)
        for sub in range(4):
            t = 4 * g + sub
            for ki in range(KI):
                nc.tensor.matmul(
                    psv[:, sub, :],
                    xt[:, ki, t * 128:(t + 1) * 128],
                    w_sb[:, 2, ki, :],
                    start=(ki == 0), stop=(ki == KI - 1),
                )
        for h in range(HPC):
            for sub in range(4):
                t = 4 * g + sub
                nc.vector.tensor_copy(
                    v_sb[:, h, t, 0:D], psv[:, sub, 64 * h:64 * h + 64])

    v_group(0, "psA")
    v_group(1, "psB")

    # embeds on DVE (bf16 SBUF 2x mode)
    for h in range(HPC):
        r = slice(64 * h, 64 * h + 64)
        for t in range(2):  # 0=cos part, 1=sin part
            e = slice(64 * t, 64 * t + 64)
            nc.vector.tensor_mul(emb_q[h][e, :], qk_mag[0][r, :], tq_sb[r, t, :])
            nc.vector.tensor_mul(emb_k[h][e, :], qk_mag[1][r, :], tk_sb[r, t, :])

    # ---- phase D: attention, (head, key-half) pipelined ----
    sc_ps = ps_pool.tile([128, S], dt.float32, name="sc_ps", tag="psA")
    av_ps = [None, None]
    exp_tiles = {}

    def scores_chunk(h, kc):
        e = exp_pool.tile([128, S], dt.bfloat16,
                          name=f"exp{h}_{kc}", tag=f"exp{kc % 6}")
        exp_tiles[(h, kc)] = e
        for qc in range(QC):
            nc.tensor.matmul(
                sc_ps[:, qc * 512:(qc + 1) * 512],
                emb_k[h][:, kc * 128:(kc + 1) * 128],
                emb_q[h][:, qc * 512:(qc + 1) * 512],
                start=True, stop=True,
            )
        nc.scalar.activation(e[:], sc_ps[:], AF.Exp, scale=1.0 / math.sqrt(D))

    def av_chunk(h, kc):
        pav = av_ps[h]
        e = exp_tiles[(h, kc)]
        for qc in range(QC):
            nc.tensor.matmul(
                pav[0:D + 1, qc * 512:(qc + 1) * 512],
                v_sb[:, h, kc, :],
                e[:, qc * 512:(qc + 1) * 512],
                start=(kc == 0), stop=(kc == KC - 1),
            )

    def normalize(h):
        # Evict the [65, S] attn@v accumulator to SBUF immediately so the
        # PSUM slot frees fast; then 1/rowsum via DMA spread across 128
        # partitions (DVE reciprocal is free-size bound ~6.3 ns/elem/lane:
        # [1,2048] costs 13us, [128,16] ~0.1us), gpsimd broadcast, multiply.
        pav = av_ps[h]
        acopy = persist.tile([D + 1, S], dt.float32, name=f"acopy{h}", tag="acopy")
        nc.scalar.activation(acopy[:], pav[0:D + 1, :], AF.Copy)
        rs128 = persist.tile([128, S // 128], dt.float32,
                             name=f"rs128_{h}", tag="rs128")
        nc.sync.dma_start(rs128[:], acopy[D:D + 1, :])
        rr128 = persist.tile([128, S // 128], dt.bfloat16,
                             name=f"rr128_{h}", tag="rr128")
        with nc.allow_low_precision(reason="softmax 1/rowsum in bf16 is ~0.4% scale noise, well within tolerance"):
            nc.vector.reciprocal(rr128[:], rs128[:])
        rr = persist.tile([1, S], dt.bfloat16, name=f"rr{h}", tag="rr")
        nc.sync.dma_start(rr[:], rr128[:])
        rsb = persist.tile([64, S], dt.bfloat16, name=f"rsb{h}", tag="rsb")
        nc.gpsimd.partition_broadcast(rsb[:], rr[:])
        nc.vector.tensor_mul(outT[64 * h:64 * h + 64, :], acopy[0:D, :], rsb[:])

    # In each iteration the ACT-independent work (v-projection / attn@v of
    # the previous stage) is emitted BEFORE the scores chunk: TensorE runs
    # in order, and the scores chunk stalls on the previous exp eviction.
    # stage 0: scores/exp (h0, 1st half) + v-projection groups 2-3
    for j in range(8):
        if j == 0:
            v_group(2, "psB")
        if j == 3:
            v_group(3, "psB")
        scores_chunk(0, j)
    # stage 1: scores/exp (h0, 2nd half) + av (h0, 1st half)
    av_ps[0] = ps_pool.tile([128, S], dt.float32, name="av0", tag="psB")
    for j in range(8):
        av_chunk(0, j)
        scores_chunk(0, 8 + j)
    # stage 2: scores/exp (h1, 1st half) + av (h0, 2nd half)
    for j in range(8):
        av_chunk(0, 8 + j)
        scores_chunk(1, j)
    normalize(0)
    # stage 3: scores/exp (h1, 2nd half) + av (h1, 1st half, lagged so
    # normalize(0)'s eviction can release the psB slot first)
    av_ps[1] = ps_pool.tile([128, S], dt.float32, name="av1", tag="psB")
    LAG = 2
    for j in range(8):
        if j >= LAG:
            av_chunk(1, j - LAG)
        scores_chunk(1, 8 + j)
    for kc in range(8 - LAG, KC):
        av_chunk(1, kc)
    normalize(1)

    # ---- phase E: partial output projection ----
    for oc in range(OC):
        psy = ps_pool.tile([128, S], dt.float32, name=f"psy{oc}",
                           tag=("psA", "psB")[oc % 2])
        for qc in range(QC):
            nc.tensor.matmul(
                psy[:, qc * 512:(qc + 1) * 512],
                wo_sb[:, oc * 128:(oc + 1) * 128],
                outT[:, qc * 512:(qc + 1) * 512],
                start=True, stop=True,
            )
        y_sb = out_pool.tile([128, S], dt.bfloat16, name=f"y{oc}", tag="y")
        # split each eviction across ACT and DVE halves (both engines idle here)
        nc.scalar.activation(y_sb[:, 0:S // 2], psy[:, 0:S // 2], AF.Copy)
        nc.vector.tensor_copy(y_sb[:, S // 2:S], psy[:, S // 2:S])
        nc.sync.dma_start(y_ext[oc, :, :], y_sb[:])


def _build():
    nc = bacc.Bacc()
    dt = mybir.dt

    ext = (
        nc.declare_dram_parameter("xt", [128, KI, S], dt.bfloat16, isOutput=False),
        nc.declare_dram_parameter("w", [128, 3, KI, ED], dt.bfloat16, isOutput=False),
        nc.declare_dram_parameter("trig_q", [128, 2, S], dt.bfloat16, isOutput=False),
        nc.declare_dram_parameter("trig_k", [128, 2, S], dt.bfloat16, isOutput=False),
        nc.declare_dram_parameter("woT", [128, DIM], dt.bfloat16, isOutput=False),
        nc.declare_dram_parameter("yT", [OC, 128, S], dt.bfloat16, isOutput=True),
    )

    with tile.TileContext(nc) as tc:
        with tc.tile_pool(name="persist", bufs=1) as persist, \
             tc.tile_pool(name="ps", bufs=1, space="PSUM") as ps_pool, \
             tc.tile_pool(name="out", bufs=2) as out_pool, \
             tc.tile_pool(name="xtp", bufs=1) as xt_pool, \
             tc.tile_pool(name="expp", bufs=2) as exp_pool:
            _build_body(nc, tc, persist, ps_pool, out_pool, xt_pool, exp_pool, ext)
    nc.compile()
    return nc


def _get_nc():
    global _compiled_nc
    if _compiled_nc is None:
        _compiled_nc = _build()
    return _compiled_nc


def _prep_inputs(x, wq, wk, wv, wo, pope_bias):
    """Host-side sharding + layout prep. Returns in_maps for the 8 cores."""
    x2 = np.ascontiguousarray(x.reshape(S, DIM).astype(np.float32))

    # trig tables (f64 phases for accuracy)
    inv = 10000.0 ** (-(np.arange(D, dtype=np.float64) / D))
    pos = np.arange(S, dtype=np.float64)
    freqs = pos[:, None] * inv[None, :]                       # [S, D]
    bias = np.clip(pope_bias.astype(np.float64), -2 * math.pi, 0.0)  # [H, D]

    cos_q = np.cos(freqs).T.astype(BF16)                      # [D, S]
    sin_q = np.sin(freqs).T.astype(BF16)
    trig_q = np.empty((128, 2, S), BF16)
    trig_q[0:64, 0] = cos_q
    trig_q[64:128, 0] = cos_q
    trig_q[0:64, 1] = sin_q
    trig_q[64:128, 1] = sin_q

    # xt[q, ki, s] = x[s, ki*128+q]
    xt = np.ascontiguousarray(
        x2.T.reshape(KI, 128, S).transpose(1, 0, 2)).astype(BF16)

    in_maps = []
    for c in range(NCORES):
        hs = slice(c * HPC * D, (c + 1) * HPC * D)            # head-channel slice
        # lhsT chunk for proj p is w_p[hs].T[ki*128:(ki+1)*128, :]
        w = np.empty((128, 3, KI, ED), BF16)
        for p, wm in enumerate((wq, wk, wv)):
            wt = np.ascontiguousarray(wm[hs, :].astype(np.float32).T)  # [DIM, ED]
            w[:, p] = wt.reshape(KI, 128, ED).transpose(1, 0, 2)

        ph = freqs[None, :, :] + bias[c * HPC:(c + 1) * HPC, None, :]  # [HPC, S, D]
        trig_k = np.empty((128, 2, S), BF16)
        for h in range(HPC):
            trig_k[64 * h:64 * h + 64, 0] = np.cos(ph[h]).T
            trig_k[64 * h:64 * h + 64, 1] = np.sin(ph[h]).T

        woT = np.ascontiguousarray(wo[:, hs].astype(np.float32).T).astype(BF16)

        in_maps.append({
            "xt": xt, "w": w, "trig_q": trig_q, "trig_k": trig_k, "woT": woT,
        })
    return in_maps


def kernel(x, wq, wk, wv, wo, pope_bias):
    nc = _get_nc()
    in_maps = _prep_inputs(np.asarray(x), np.asarray(wq), np.asarray(wk),
                           np.asarray(wv), np.asarray(wo), np.asarray(pope_bias))
    res = run_bass_kernel_spmd(nc, in_maps, list(range(NCORES)))
    y = np.zeros((DIM, S), np.float32)
    for c in range(NCORES):
        y += res.results[c]["yT"].reshape(DIM, S).astype(np.float32)
    return np.ascontiguousarray(y.T).reshape(1, S, DIM)


if __name__ == "__main__":
    rng = np.random.default_rng(0)
    out = kernel(
        x=rng.standard_normal((1, S, DIM)).astype(np.float32),
        wq=rng.standard_normal((DIM, DIM)).astype(np.float32) / 32,
        wk=rng.standard_normal((DIM, DIM)).astype(np.float32) / 32,
        wv=rng.standard_normal((DIM, DIM)).astype(np.float32) / 32,
        wo=rng.standard_normal((DIM, DIM)).astype(np.float32) / 32,
        pope_bias=-rng.random((H, D), np.float32) * 3.0,
    )
    print("out", out.shape, out.dtype, np.abs(out).mean())


# revision 31
# speedup vs baseline: 1.2107x; 1.2107x over previous
"""PoPE attention kernel for Trainium2, sharded over 8 NeuronCores by heads.

Problem: B=1, S=2048, DIM=1024, H=16 heads, D=64.
  q/k/v = x @ w{q,k,v}^T ; PoPE embed (softplus magnitude x cos/sin phase);
  scores = q_emb @ k_emb^T / sqrt(D); softmax; out = attn @ v; y = out @ wo^T.

Sharding: 2 heads per core. Each core computes its heads' projections,
attention, and a partial output projection (its 128 channels of wo);
host sums the 8 partial y's (f32) - no on-chip collectives.

Layouts are "transposed" (feature-major): xT [DIM, S] so that every
matmul has its contraction on the partition axis with no on-chip
transposes. All matmuls are bf16 with f32 PSUM accumulate.
The softmax skips max-subtraction (scores/8 are bounded ~4.3), and the
rowsum comes free from a ones-column appended to v in the attn@v matmul.
Attention is pipelined in (head, half-of-keys) stages so exp buffers fit
SBUF; attn@v of the previous stage overlaps scores+exp of the current,
and the v-projection fills TensorE gaps during the first stage.
The softmax 1/rowsum uses a DMA reshape to spread the row over 128
partitions (DVE reciprocal is ~6.3ns per element per lane; [1,2048]
would cost 13us, [128,16] costs ~0.1us).
"""
import math

import numpy as np
import ml_dtypes

import concourse.bacc as bacc
import concourse.mybir as mybir
from concourse import tile
from concourse.bass_utils import run_bass_kernel_spmd

BF16 = ml_dtypes.bfloat16
S, DIM, H, D = 2048, 1024, 16, 64
NCORES = 8
HPC = H // NCORES          # heads per core = 2
ED = 2 * D                 # embedding width per head = 128
KI = DIM // 128            # contraction chunks for projections = 8
KC = S // 128              # key-token chunks = 16
QC = S // 512              # query free-dim chunks of 512 = 4
OC = DIM // 128            # output-channel chunks = 8

_compiled_nc = None


def _build_body(nc, tc, persist, ps_pool, out_pool, xt_pool, exp_pool, ext):
    dt = mybir.dt
    AF = mybir.ActivationFunctionType
    xt_ext, w_ext, tq_ext, tk_ext, wo_ext, y_ext = ext
    QH = 1024                  # query superblock width (enables PSUM
    #                            double-buffering of the score tiles)

    # ---- HAM warmup: dummy matmuls on junk data while the input DMAs run,
    # so the PE clock-gate reaches 2.4 GHz before the real matmuls start ----
    warm_sb = persist.tile([128, 512], dt.bfloat16)
    nc.gpsimd.memset(warm_sb[:], 0.0)
    warm_ps = ps_pool.tile([128, 512], dt.float32, name="warm_ps", tag="scA")
    for i in range(16):
        nc.tensor.matmul(warm_ps[:], warm_sb[:, 0:128], warm_sb[:],
                         start=(i == 0), stop=(i == 15))

    # ---- phase A: input DMAs (w and xt first so matmuls start early) ----
    w_sb = persist.tile([128, 3, KI, ED], dt.bfloat16)
    nc.sync.dma_start(w_sb[:], w_ext[:])
    xt = xt_pool.tile([128, KI, S], dt.bfloat16)
    for ki in range(KI):
        nc.sync.dma_start(xt[:, ki, :], xt_ext[:, ki, :])
    tq_sb = persist.tile([128, 2, S], dt.bfloat16)
    nc.sync.dma_start(tq_sb[:], tq_ext[:])
    tk_sb = persist.tile([128, 2, S], dt.bfloat16)
    nc.sync.dma_start(tk_sb[:], tk_ext[:])
    wo_sb = persist.tile([128, DIM], dt.bfloat16)
    nc.sync.dma_start(wo_sb[:], wo_ext[:])
    # v with a ones column appended per (head, key chunk)
    v_sb = persist.tile([128, HPC, KC, D + 1], dt.bfloat16)
    nc.gpsimd.memset(v_sb[:, 0, :, D], 1.0)
    nc.gpsimd.memset(v_sb[:, 1, :, D], 1.0)
    emb_q = [persist.tile([128, S], dt.bfloat16, name=f"embq{h}", tag=f"embq{h}")
             for h in range(HPC)]
    emb_k = [persist.tile([128, S], dt.bfloat16, name=f"embk{h}", tag=f"embk{h}")
             for h in range(HPC)]
    outT = persist.tile([128, S], dt.bfloat16)

    # PSUM layout: four 2-bank tags. Scores ping-pong on scA/scB while the
    # two attn@v accumulators sit on avA/avB; the projections and the output
    # projection reuse the same four tags.
    # ---- phase B: q/k projections (ki-outer so MMs start after the first
    # xt chunk lands), clustered softplus ----
    psm = {}
    for p in range(2):  # 0=q, 1=k
        for lo in range(2):
            t = ps_pool.tile([128, QH], dt.float32, name=f"psm{p}_{lo}",
                             tag=("scA", "avA", "scB", "avB")[2 * p + lo])
            psm[(p, lo)] = t
        for ki in range(KI):
            for qc in range(QC):
                nc.tensor.matmul(
                    psm[(p, qc // 2)][:, (qc % 2) * 512:(qc % 2) * 512 + 512],
                    w_sb[:, p, ki, :],
                    xt[:, ki, qc * 512:(qc + 1) * 512],
                    start=(ki == 0), stop=(ki == KI - 1),
                )
    # softplus(x) = ln(1 + e^x); all four Exp quarters feed ONE combined Ln
    # so the scheduler cannot interleave exp/ln (different ACT table sets;
    # each switch costs a ~1.3us table load).
    tmp = xt_pool.tile([128, 2, S], dt.float32, name="sp", tag="sp")
    mag = xt_pool.tile([128, 2, S], dt.bfloat16, name="mag", tag="mag")
    qk_mag = [mag[:, 0, :], mag[:, 1, :]]
    for p in range(2):
        for lo in range(2):
            nc.scalar.activation(tmp[:, p, lo * QH:(lo + 1) * QH],
                                 psm[(p, lo)][:], AF.Exp)
    nc.scalar.activation(mag[:], tmp[:], AF.Ln, bias=1.0)

    # ---- phase C: v projection (token-major directly) ----
    def v_group(g, tag):
        psv = ps_pool.tile([128, 4, 128], dt.float32, name=f"psv{g}", tag=tag)
        for sub in range(4):
            t = 4 * g + sub
            for ki in range(KI):
                nc.tensor.matmul(
                    psv[:, sub, :],
                    xt[:, ki, t * 128:(t + 1) * 128],
                    w_sb[:, 2, ki, :],
                    start=(ki == 0), stop=(ki == KI - 1),
                )
        for h in range(HPC):
            for sub in range(4):
                t = 4 * g + sub
                nc.vector.tensor_copy(
                    v_sb[:, h, t, 0:D], psv[:, sub, 64 * h:64 * h + 64])

    v_group(0, "scA")
    v_group(1, "scB")

    # embeds: q-side on DVE (bf16 SBUF 2x mode), k-side on gpsimd in
    # parallel, head 0 first so its score chunks can start sooner
    for h in range(HPC):
        r = slice(64 * h, 64 * h + 64)
        for t in range(2):  # 0=cos part, 1=sin part
            e = slice(64 * t, 64 * t + 64)
            nc.vector.tensor_mul(emb_q[h][e, :], qk_mag[0][r, :], tq_sb[r, t, :])
            nc.vector.tensor_mul(emb_k[h][e, :], qk_mag[1][r, :], tk_sb[r, t, :])

    # ---- phase D: attention in two query superblocks of 1024; inside each,
    # (head, key-half) stages. Score tiles double-buffer on scA/scB so the
    # next chunk's matmuls overlap the previous chunk's exp eviction. ----
    av_ps = {}
    exp_tiles = {}

    def scores_chunk(h, kc, qh):
        e = exp_pool.tile([128, QH], dt.bfloat16,
                          name=f"exp{qh}_{h}_{kc}", tag=f"exp{qh}_{kc % 8}")
        exp_tiles[(h, kc, qh)] = e
        sc = ps_pool.tile([128, QH], dt.float32, name=f"sc{qh}_{h}_{kc}",
                          tag=("scA", "scB")[kc % 2])
        for q2 in range(2):
            nc.tensor.matmul(
                sc[:, q2 * 512:(q2 + 1) * 512],
                emb_k[h][:, kc * 128:(kc + 1) * 128],
                emb_q[h][:, qh * QH + q2 * 512:qh * QH + (q2 + 1) * 512],
                start=True, stop=True,
            )
        nc.scalar.activation(e[:], sc[:], AF.Exp, scale=1.0 / math.sqrt(D))

    def av_chunk(h, kc, qh):
        pav = av_ps[(h, qh)]
        e = exp_tiles[(h, kc, qh)]
        for q2 in range(2):
            nc.tensor.matmul(
                pav[0:D + 1, q2 * 512:(q2 + 1) * 512],
                v_sb[:, h, kc, :],
                e[:, q2 * 512:(q2 + 1) * 512],
                start=(kc == 0), stop=(kc == KC - 1),
            )

    def normalize(h, qh):
        # Evict the [65, QH] attn@v accumulator to SBUF immediately so the
        # PSUM slot frees fast; then 1/rowsum via DMA spread across 128
        # partitions (DVE reciprocal is free-size bound ~6.3 ns/elem/lane),
        # gpsimd broadcast, multiply.
        pav = av_ps[(h, qh)]
        acopy = persist.tile([D + 1, QH], dt.float32,
                             name=f"acopy{h}_{qh}", tag=f"acopy{h}")
        nc.scalar.activation(acopy[:], pav[0:D + 1, :], AF.Copy)
        rs128 = persist.tile([128, QH // 128], dt.float32,
                             name=f"rs128_{h}_{qh}", tag="rs128")
        nc.sync.dma_start(rs128[:], acopy[D:D + 1, :])
        rr128 = persist.tile([128, QH // 128], dt.bfloat16,
                             name=f"rr128_{h}_{qh}", tag="rr128")
        with nc.allow_low_precision(reason="softmax 1/rowsum in bf16 is ~0.4% scale noise"):
            nc.vector.reciprocal(rr128[:], rs128[:])
        rr = persist.tile([1, QH], dt.bfloat16, name=f"rr{h}_{qh}", tag="rr")
        nc.sync.dma_start(rr[:], rr128[:])
        rsb = persist.tile([64, QH], dt.bfloat16, name=f"rsb{h}_{qh}", tag="rsb")
        nc.gpsimd.partition_broadcast(rsb[:], rr[:])
        nc.vector.tensor_mul(outT[64 * h:64 * h + 64, qh * QH:(qh + 1) * QH],
                             acopy[0:D, :], rsb[:])

    def oproj(qh, oc, tags=("avA", "avB")):
        c = slice(qh * QH, (qh + 1) * QH)
        psy = ps_pool.tile([128, QH], dt.float32, name=f"psy{qh}_{oc}",
                           tag=tags[oc % len(tags)])
        for q2 in range(2):
            nc.tensor.matmul(
                psy[:, q2 * 512:(q2 + 1) * 512],
                wo_sb[:, oc * 128:(oc + 1) * 128],
                outT[:, qh * QH + q2 * 512:qh * QH + (q2 + 1) * 512],
                start=True, stop=True,
            )
        y_sb = out_pool.tile([128, QH], dt.bfloat16, name=f"y{qh}_{oc}", tag="y")
        # split the eviction across ACT and DVE halves
        nc.scalar.activation(y_sb[:, 0:QH // 2], psy[:, 0:QH // 2], AF.Copy)
        nc.vector.tensor_copy(y_sb[:, QH // 2:QH], psy[:, QH // 2:QH])
        nc.sync.dma_start(y_ext[oc, :, c], y_sb[:])

    LAG = 1
    for qh in range(2):
        # stage 0: scores/exp (h0, 1st key half); fillers keep PE dense:
        # superblock 0 runs the last v-projection groups, superblock 1 runs
        # superblock 0's output projection on the freed avA/avB banks.
        for j in range(8):
            if qh == 0:
                if j == 0:
                    v_group(2, "avA")
                if j == 3:
                    v_group(3, "avB")
            if qh == 1:
                oproj(0, j)
            scores_chunk(0, j, qh)
        # stage 1: scores/exp (h0, 2nd half) + av (h0, 1st half)
        av_ps[(0, qh)] = ps_pool.tile([128, QH], dt.float32,
                                      name=f"av0_{qh}", tag="avA")
        for j in range(8):
            av_chunk(0, j, qh)
            scores_chunk(0, 8 + j, qh)
        # stage 2: scores/exp (h1, 1st half) + av (h0, 2nd half)
        av_ps[(1, qh)] = ps_pool.tile([128, QH], dt.float32,
                                      name=f"av1_{qh}", tag="avB")
        for j in range(8):
            av_chunk(0, 8 + j, qh)
            scores_chunk(1, j, qh)
        normalize(0, qh)
        # stage 3: scores/exp (h1, 2nd half) + av (h1, 1st half)
        for j in range(8):
            if j >= LAG:
                av_chunk(1, j - LAG, qh)
            scores_chunk(1, 8 + j, qh)
        for kc in range(8 - LAG, KC):
            av_chunk(1, kc, qh)
        normalize(1, qh)

    # ---- phase E: output projection for the last superblock ----
    for oc in range(OC):
        oproj(1, oc)


def _build():
    nc = bacc.Bacc()
    dt = mybir.dt

    ext = (
        nc.declare_dram_parameter("xt", [128, KI, S], dt.bfloat16, isOutput=False),
        nc.declare_dram_parameter("w", [128, 3, KI, ED], dt.bfloat16, isOutput=False),
        nc.declare_dram_parameter("trig_q", [128, 2, S], dt.bfloat16, isOutput=False),
        nc.declare_dram_parameter("trig_k", [128, 2, S], dt.bfloat16, isOutput=False),
        nc.declare_dram_parameter("woT", [128, DIM], dt.bfloat16, isOutput=False),
        nc.declare_dram_parameter("yT", [OC, 128, S], dt.bfloat16, isOutput=True),
    )

    with tile.TileContext(nc) as tc:
        with tc.tile_pool(name="persist", bufs=1) as persist, \
             tc.tile_pool(name="ps", bufs=1, space="PSUM") as ps_pool, \
             tc.tile_pool(name="out", bufs=2) as out_pool, \
             tc.tile_pool(name="xtp", bufs=1) as xt_pool, \
             tc.tile_pool(name="expp", bufs=2) as exp_pool:
            _build_body(nc, tc, persist, ps_pool, out_pool, xt_pool, exp_pool, ext)
    nc.compile()
    return nc


def _get_nc():
    global _compiled_nc
    if _compiled_nc is None:
        _compiled_nc = _build()
    return _compiled_nc


def _prep_inputs(x, wq, wk, wv, wo, pope_bias):
    """Host-side sharding + layout prep. Returns in_maps for the 8 cores."""
    x2 = np.ascontiguousarray(x.reshape(S, DIM).astype(np.float32))

    # trig tables (f64 phases for accuracy)
    inv = 10000.0 ** (-(np.arange(D, dtype=np.float64) / D))
    pos = np.arange(S, dtype=np.float64)
    freqs = pos[:, None] * inv[None, :]                       # [S, D]
    bias = np.clip(pope_bias.astype(np.float64), -2 * math.pi, 0.0)  # [H, D]

    cos_q = np.cos(freqs).T.astype(BF16)                      # [D, S]
    sin_q = np.sin(freqs).T.astype(BF16)
    trig_q = np.empty((128, 2, S), BF16)
    trig_q[0:64, 0] = cos_q
    trig_q[64:128, 0] = cos_q
    trig_q[0:64, 1] = sin_q
    trig_q[64:128, 1] = sin_q

    # xt[q, ki, s] = x[s, ki*128+q]
    xt = np.ascontiguousarray(
        x2.T.reshape(KI, 128, S).transpose(1, 0, 2)).astype(BF16)

    in_maps = []
    for c in range(NCORES):
        hs = slice(c * HPC * D, (c + 1) * HPC * D)            # head-channel slice
        # lhsT chunk for proj p is w_p[hs].T[ki*128:(ki+1)*128, :]
        w = np.empty((128, 3, KI, ED), BF16)
        for p, wm in enumerate((wq, wk, wv)):
            wt = np.ascontiguousarray(wm[hs, :].astype(np.float32).T)  # [DIM, ED]
            w[:, p] = wt.reshape(KI, 128, ED).transpose(1, 0, 2)

        ph = freqs[None, :, :] + bias[c * HPC:(c + 1) * HPC, None, :]  # [HPC, S, D]
        trig_k = np.empty((128, 2, S), BF16)
        for h in range(HPC):
            trig_k[64 * h:64 * h + 64, 0] = np.cos(ph[h]).T
            trig_k[64 * h:64 * h + 64, 1] = np.sin(ph[h]).T

        woT = np.ascontiguousarray(wo[:, hs].astype(np.float32).T).astype(BF16)

        in_maps.append({
            "xt": xt, "w": w, "trig_q": trig_q, "trig_k": trig_k, "woT": woT,
        })
    return in_maps


def kernel(x, wq, wk, wv, wo, pope_bias):
    nc = _get_nc()
    in_maps = _prep_inputs(np.asarray(x), np.asarray(wq), np.asarray(wk),
                           np.asarray(wv), np.asarray(wo), np.asarray(pope_bias))
    res = run_bass_kernel_spmd(nc, in_maps, list(range(NCORES)))
    y = np.zeros((DIM, S), np.float32)
    for c in range(NCORES):
        y += res.results[c]["yT"].reshape(DIM, S).astype(np.float32)
    return np.ascontiguousarray(y.T).reshape(1, S, DIM)


if __name__ == "__main__":
    rng = np.random.default_rng(0)
    out = kernel(
        x=rng.standard_normal((1, S, DIM)).astype(np.float32),
        wq=rng.standard_normal((DIM, DIM)).astype(np.float32) / 32,
        wk=rng.standard_normal((DIM, DIM)).astype(np.float32) / 32,
        wv=rng.standard_normal((DIM, DIM)).astype(np.float32) / 32,
        wo=rng.standard_normal((DIM, DIM)).astype(np.float32) / 32,
        pope_bias=-rng.random((H, D), np.float32) * 3.0,
    )
    print("out", out.shape, out.dtype, np.abs(out).mean())


# revision 39
# speedup vs baseline: 1.2353x; 1.0203x over previous
"""PoPE attention kernel for Trainium2, sharded over 8 NeuronCores by heads.

Problem: B=1, S=2048, DIM=1024, H=16 heads, D=64.
  q/k/v = x @ w{q,k,v}^T ; PoPE embed (softplus magnitude x cos/sin phase);
  scores = q_emb @ k_emb^T / sqrt(D); softmax; out = attn @ v; y = out @ wo^T.

Sharding: 2 heads per core. Each core computes its heads' projections,
attention, and a partial output projection (its 128 channels of wo);
host sums the 8 partial y's (f32) - no on-chip collectives.

Layouts are "transposed" (feature-major): xT [DIM, S] so that every
matmul has its contraction on the partition axis with no on-chip
transposes. All matmuls are bf16 with f32 PSUM accumulate.
The softmax skips max-subtraction (scores/8 are bounded ~4.3), and the
rowsum comes free from a ones-column appended to v in the attn@v matmul.
Attention is pipelined in (head, half-of-keys) stages so exp buffers fit
SBUF; attn@v of the previous stage overlaps scores+exp of the current,
and the v-projection fills TensorE gaps during the first stage.
The softmax 1/rowsum uses a DMA reshape to spread the row over 128
partitions (DVE reciprocal is ~6.3ns per element per lane; [1,2048]
would cost 13us, [128,16] costs ~0.1us).
"""
import math

import numpy as np
import ml_dtypes

import concourse.bacc as bacc
import concourse.mybir as mybir
from concourse import tile
from concourse.bass_utils import run_bass_kernel_spmd

BF16 = ml_dtypes.bfloat16
S, DIM, H, D = 2048, 1024, 16, 64
NCORES = 8
HPC = H // NCORES          # heads per core = 2
ED = 2 * D                 # embedding width per head = 128
KI = DIM // 128            # contraction chunks for projections = 8
KC = S // 128              # key-token chunks = 16
QC = S // 512              # query free-dim chunks of 512 = 4
OC = DIM // 128            # output-channel chunks = 8

_compiled_nc = None


def _build_body(nc, tc, persist, ps_pool, out_pool, xt_pool, exp_pool, ext):
    dt = mybir.dt
    AF = mybir.ActivationFunctionType
    xt_ext, w_ext, tq_ext, tk_ext, wo_ext, y_ext = ext
    QH = 1024                  # query superblock width (enables PSUM
    #                            double-buffering of the score tiles)

    # ---- HAM warmup: dummy matmuls on junk data while the input DMAs run,
    # so the PE clock-gate reaches 2.4 GHz before the real matmuls start ----
    warm_sb = persist.tile([128, 512], dt.bfloat16)
    nc.gpsimd.memset(warm_sb[:], 0.0)
    warm_ps = ps_pool.tile([128, 512], dt.float32, name="warm_ps", tag="scA")
    for i in range(16):
        nc.tensor.matmul(warm_ps[:], warm_sb[:, 0:128], warm_sb[:],
                         start=(i == 0), stop=(i == 15))

    # ---- phase A: input DMAs (w and xt first so matmuls start early) ----
    w_sb = persist.tile([128, 3, KI, ED], dt.bfloat16)
    nc.sync.dma_start(w_sb[:], w_ext[:])
    xt = xt_pool.tile([128, KI, S], dt.bfloat16)
    for ki in range(KI):
        nc.sync.dma_start(xt[:, ki, :], xt_ext[:, ki, :])
    tq_sb = persist.tile([128, 2, S], dt.bfloat16)
    nc.sync.dma_start(tq_sb[:], tq_ext[:])
    tk_sb = persist.tile([128, 2, S], dt.bfloat16)
    nc.sync.dma_start(tk_sb[:], tk_ext[:])
    wo_sb = persist.tile([128, DIM], dt.bfloat16)
    nc.sync.dma_start(wo_sb[:], wo_ext[:])
    # v with a ones column appended per (head, key chunk)
    v_sb = persist.tile([128, HPC, KC, D + 1], dt.bfloat16)
    nc.gpsimd.memset(v_sb[:, 0, :, D], 1.0)
    nc.gpsimd.memset(v_sb[:, 1, :, D], 1.0)
    emb_q = [persist.tile([128, S], dt.bfloat16, name=f"embq{h}", tag=f"embq{h}")
             for h in range(HPC)]
    emb_k = [persist.tile([128, S], dt.bfloat16, name=f"embk{h}", tag=f"embk{h}")
             for h in range(HPC)]
    outT = persist.tile([128, S], dt.bfloat16)

    # PSUM layout: four 2-bank tags. Scores ping-pong on scA/scB while the
    # two attn@v accumulators sit on avA/avB; the projections and the output
    # projection reuse the same four tags.
    # ---- phase B: q/k projections (ki-outer so MMs start after the first
    # xt chunk lands), clustered softplus ----
    psm = {}
    for p in range(2):  # 0=q, 1=k
        for lo in range(2):
            t = ps_pool.tile([128, QH], dt.float32, name=f"psm{p}_{lo}",
                             tag=("scA", "avA", "scB", "avB")[2 * p + lo])
            psm[(p, lo)] = t
        for ki in range(KI):
            for qc in range(QC):
                nc.tensor.matmul(
                    psm[(p, qc // 2)][:, (qc % 2) * 512:(qc % 2) * 512 + 512],
                    w_sb[:, p, ki, :],
                    xt[:, ki, qc * 512:(qc + 1) * 512],
                    start=(ki == 0), stop=(ki == KI - 1),
                )
    # softplus(x) = ln(1 + e^x); all four Exp quarters feed ONE combined Ln
    # so the scheduler cannot interleave exp/ln (different ACT table sets;
    # each switch costs a ~1.3us table load).
    tmp = xt_pool.tile([128, 2, S], dt.float32, name="sp", tag="sp")
    mag = xt_pool.tile([128, 2, S], dt.bfloat16, name="mag", tag="mag")
    qk_mag = [mag[:, 0, :], mag[:, 1, :]]
    for p in range(2):
        for lo in range(2):
            nc.scalar.activation(tmp[:, p, lo * QH:(lo + 1) * QH],
                                 psm[(p, lo)][:], AF.Exp)
    nc.scalar.activation(mag[:, :, 0:QH], tmp[:, :, 0:QH], AF.Ln, bias=1.0)
    nc.scalar.activation(mag[:, :, QH:S], tmp[:, :, QH:S], AF.Ln, bias=1.0)

    # ---- phase C: v projection (token-major directly) ----
    def v_group(g, tag):
        psv = ps_pool.tile([128, 4, 128], dt.float32, name=f"psv{g}", tag=tag)
        for sub in range(4):
            t = 4 * g + sub
            for ki in range(KI):
                nc.tensor.matmul(
                    psv[:, sub, :],
                    xt[:, ki, t * 128:(t + 1) * 128],
                    w_sb[:, 2, ki, :],
                    start=(ki == 0), stop=(ki == KI - 1),
                )
        for h in range(HPC):
            for sub in range(4):
                t = 4 * g + sub
                nc.vector.tensor_copy(
                    v_sb[:, h, t, 0:D], psv[:, sub, 64 * h:64 * h + 64])

    v_group(0, "scA")
    v_group(1, "scB")

    # embeds on DVE (bf16 SBUF 2x mode), in column halves with head 0's
    # low half first so its first score chunks can start sooner
    for lo in range(2):
        c = slice(lo * QH, (lo + 1) * QH)
        for h in range(HPC):
            r = slice(64 * h, 64 * h + 64)
            for t in range(2):  # 0=cos part, 1=sin part
                e = slice(64 * t, 64 * t + 64)
                nc.vector.tensor_mul(emb_q[h][e, c], qk_mag[0][r, c], tq_sb[r, t, c])
                nc.vector.tensor_mul(emb_k[h][e, c], qk_mag[1][r, c], tk_sb[r, t, c])

    # ---- phase D: attention in two query superblocks of 1024; inside each,
    # (head, key-half) stages. Score tiles double-buffer on scA/scB so the
    # next chunk's matmuls overlap the previous chunk's exp eviction. ----
    av_ps = {}
    exp_tiles = {}

    def scores_chunk(h, kc, qh):
        e = exp_pool.tile([128, QH], dt.bfloat16,
                          name=f"exp{qh}_{h}_{kc}", tag=f"exp{qh}_{kc % 8}")
        exp_tiles[(h, kc, qh)] = e
        sc = ps_pool.tile([128, QH], dt.float32, name=f"sc{qh}_{h}_{kc}",
                          tag=("scA", "scB")[kc % 2])
        for q2 in range(2):
            nc.tensor.matmul(
                sc[:, q2 * 512:(q2 + 1) * 512],
                emb_k[h][:, kc * 128:(kc + 1) * 128],
                emb_q[h][:, qh * QH + q2 * 512:qh * QH + (q2 + 1) * 512],
                start=True, stop=True,
            )
        nc.scalar.activation(e[:], sc[:], AF.Exp, scale=1.0 / math.sqrt(D))

    def av_chunk(h, kc, qh):
        pav = av_ps[(h, qh)]
        e = exp_tiles[(h, kc, qh)]
        for q2 in range(2):
            nc.tensor.matmul(
                pav[0:D + 1, q2 * 512:(q2 + 1) * 512],
                v_sb[:, h, kc, :],
                e[:, q2 * 512:(q2 + 1) * 512],
                start=(kc == 0), stop=(kc == KC - 1),
            )

    def normalize(h, qh):
        # Evict the [65, QH] attn@v accumulator to SBUF immediately so the
        # PSUM slot frees fast; then 1/rowsum via DMA spread across 128
        # partitions (DVE reciprocal is free-size bound ~6.3 ns/elem/lane),
        # gpsimd broadcast, multiply.
        pav = av_ps[(h, qh)]
        acopy = persist.tile([D + 1, QH], dt.float32,
                             name=f"acopy{h}_{qh}", tag=f"acopy{h}")
        nc.scalar.activation(acopy[:], pav[0:D + 1, :], AF.Copy)
        rs128 = persist.tile([128, QH // 128], dt.float32,
                             name=f"rs128_{h}_{qh}", tag="rs128")
        nc.sync.dma_start(rs128[:], acopy[D:D + 1, :])
        rr128 = persist.tile([128, QH // 128], dt.bfloat16,
                             name=f"rr128_{h}_{qh}", tag="rr128")
        with nc.allow_low_precision(reason="softmax 1/rowsum in bf16 is ~0.4% scale noise"):
            nc.vector.reciprocal(rr128[:], rs128[:])
        rr = persist.tile([1, QH], dt.bfloat16, name=f"rr{h}_{qh}", tag="rr")
        nc.sync.dma_start(rr[:], rr128[:])
        rsb = persist.tile([64, QH], dt.bfloat16, name=f"rsb{h}_{qh}", tag="rsb")
        nc.gpsimd.partition_broadcast(rsb[:], rr[:])
        nc.vector.tensor_mul(outT[64 * h:64 * h + 64, qh * QH:(qh + 1) * QH],
                             acopy[0:D, :], rsb[:])

    def oproj(qh, oc, tags=("avA", "avB")):
        c = slice(qh * QH, (qh + 1) * QH)
        psy = ps_pool.tile([128, QH], dt.float32, name=f"psy{qh}_{oc}",
                           tag=tags[oc % len(tags)])
        for q2 in range(2):
            nc.tensor.matmul(
                psy[:, q2 * 512:(q2 + 1) * 512],
                wo_sb[:, oc * 128:(oc + 1) * 128],
                outT[:, qh * QH + q2 * 512:qh * QH + (q2 + 1) * 512],
                start=True, stop=True,
            )
        y_sb = out_pool.tile([128, QH], dt.bfloat16, name=f"y{qh}_{oc}", tag="y")
        # split the eviction across ACT and DVE halves
        nc.scalar.activation(y_sb[:, 0:QH // 2], psy[:, 0:QH // 2], AF.Copy)
        nc.vector.tensor_copy(y_sb[:, QH // 2:QH], psy[:, QH // 2:QH])
        nc.sync.dma_start(y_ext[oc, :, c], y_sb[:])

    def oproj_h0(qh, oc, tag):
        # first half of the contraction (head 0's 64 channels); runs before
        # the last normalize, filling the PE idle window
        psy = ps_pool.tile([128, QH], dt.float32, name=f"psyS{qh}_{oc}", tag=tag)
        for q2 in range(2):
            nc.tensor.matmul(
                psy[:, q2 * 512:(q2 + 1) * 512],
                wo_sb[0:64, oc * 128:(oc + 1) * 128],
                outT[0:64, qh * QH + q2 * 512:qh * QH + (q2 + 1) * 512],
                start=True, stop=False,
            )
        return psy

    def oproj_h1(qh, oc, psy):
        c = slice(qh * QH, (qh + 1) * QH)
        for q2 in range(2):
            nc.tensor.matmul(
                psy[:, q2 * 512:(q2 + 1) * 512],
                wo_sb[64:128, oc * 128:(oc + 1) * 128],
                outT[64:128, qh * QH + q2 * 512:qh * QH + (q2 + 1) * 512],
                start=False, stop=True,
            )
        y_sb = out_pool.tile([128, QH], dt.bfloat16, name=f"yS{qh}_{oc}", tag="y")
        nc.scalar.activation(y_sb[:, 0:QH // 2], psy[:, 0:QH // 2], AF.Copy)
        nc.vector.tensor_copy(y_sb[:, QH // 2:QH], psy[:, QH // 2:QH])
        nc.sync.dma_start(y_ext[oc, :, c], y_sb[:])

    LAG = 1
    for qh in range(2):
        # stage 0: scores/exp (h0, 1st key half); fillers keep PE dense:
        # superblock 0 runs the last v-projection groups, superblock 1 runs
        # superblock 0's output projection on the freed avA/avB banks.
        for j in range(8):
            if qh == 0:
                if j == 0:
                    v_group(2, "avA")
                if j == 3:
                    v_group(3, "avB")
            if qh == 1:
                oproj(0, j)
            scores_chunk(0, j, qh)
        # stage 1: scores/exp (h0, 2nd half) + av (h0, 1st half)
        av_ps[(0, qh)] = ps_pool.tile([128, QH], dt.float32,
                                      name=f"av0_{qh}", tag="avA")
        for j in range(8):
            av_chunk(0, j, qh)
            scores_chunk(0, 8 + j, qh)
        # stage 2: scores/exp (h1, 1st half) + av (h0, 2nd half)
        av_ps[(1, qh)] = ps_pool.tile([128, QH], dt.float32,
                                      name=f"av1_{qh}", tag="avB")
        for j in range(8):
            av_chunk(0, 8 + j, qh)
            scores_chunk(1, j, qh)
        normalize(0, qh)
        # stage 3: scores/exp (h1, 2nd half) + av (h1, 1st half)
        for j in range(8):
            if j >= LAG:
                av_chunk(1, j - LAG, qh)
            scores_chunk(1, 8 + j, qh)
        for kc in range(8 - LAG, KC):
            av_chunk(1, kc, qh)
        normalize(1, qh)

    # ---- phase E: output projection for the last superblock (scA/scB are
    # free after the last exp -> 4-deep psy pipeline) ----
    for oc in range(OC):
        oproj(1, oc, tags=("avA", "avB", "scA", "scB"))


def _build():
    nc = bacc.Bacc()
    dt = mybir.dt

    ext = (
        nc.declare_dram_parameter("xt", [128, KI, S], dt.bfloat16, isOutput=False),
        nc.declare_dram_parameter("w", [128, 3, KI, ED], dt.bfloat16, isOutput=False),
        nc.declare_dram_parameter("trig_q", [128, 2, S], dt.bfloat16, isOutput=False),
        nc.declare_dram_parameter("trig_k", [128, 2, S], dt.bfloat16, isOutput=False),
        nc.declare_dram_parameter("woT", [128, DIM], dt.bfloat16, isOutput=False),
        nc.declare_dram_parameter("yT", [OC, 128, S], dt.bfloat16, isOutput=True),
    )

    with tile.TileContext(nc) as tc:
        with tc.tile_pool(name="persist", bufs=1) as persist, \
             tc.tile_pool(name="ps", bufs=1, space="PSUM") as ps_pool, \
             tc.tile_pool(name="out", bufs=2) as out_pool, \
             tc.tile_pool(name="xtp", bufs=1) as xt_pool, \
             tc.tile_pool(name="expp", bufs=2) as exp_pool:
            _build_body(nc, tc, persist, ps_pool, out_pool, xt_pool, exp_pool, ext)
    nc.compile()
    return nc


def _get_nc():
    global _compiled_nc
    if _compiled_nc is None:
        _compiled_nc = _build()
    return _compiled_nc


def _prep_inputs(x, wq, wk, wv, wo, pope_bias):
    """Host-side sharding + layout prep. Returns in_maps for the 8 cores."""
    x2 = np.ascontiguousarray(x.reshape(S, DIM).astype(np.float32))

    # trig tables (f64 phases for accuracy)
    inv = 10000.0 ** (-(np.arange(D, dtype=np.float64) / D))
    pos = np.arange(S, dtype=np.float64)
    freqs = pos[:, None] * inv[None, :]                       # [S, D]
    bias = np.clip(pope_bias.astype(np.float64), -2 * math.pi, 0.0)  # [H, D]

    cos_q = np.cos(freqs).T.astype(BF16)                      # [D, S]
    sin_q = np.sin(freqs).T.astype(BF16)
    trig_q = np.empty((128, 2, S), BF16)
    trig_q[0:64, 0] = cos_q
    trig_q[64:128, 0] = cos_q
    trig_q[0:64, 1] = sin_q
    trig_q[64:128, 1] = sin_q

    # xt[q, ki, s] = x[s, ki*128+q]
    xt = np.ascontiguousarray(
        x2.T.reshape(KI, 128, S).transpose(1, 0, 2)).astype(BF16)

    in_maps = []
    for c in range(NCORES):
        hs = slice(c * HPC * D, (c + 1) * HPC * D)            # head-channel slice
        # lhsT chunk for proj p is w_p[hs].T[ki*128:(ki+1)*128, :]
        w = np.empty((128, 3, KI, ED), BF16)
        for p, wm in enumerate((wq, wk, wv)):
            wt = np.ascontiguousarray(wm[hs, :].astype(np.float32).T)  # [DIM, ED]
            w[:, p] = wt.reshape(KI, 128, ED).transpose(1, 0, 2)

        ph = freqs[None, :, :] + bias[c * HPC:(c + 1) * HPC, None, :]  # [HPC, S, D]
        trig_k = np.empty((128, 2, S), BF16)
        for h in range(HPC):
            trig_k[64 * h:64 * h + 64, 0] = np.cos(ph[h]).T
            trig_k[64 * h:64 * h + 64, 1] = np.sin(ph[h]).T

        woT = np.ascontiguousarray(wo[:, hs].astype(np.float32).T).astype(BF16)

        in_maps.append({
            "xt": xt, "w": w, "trig_q": trig_q, "trig_k": trig_k, "woT": woT,
        })
    return in_maps


def kernel(x, wq, wk, wv, wo, pope_bias):
    nc = _get_nc()
    in_maps = _prep_inputs(np.asarray(x), np.asarray(wq), np.asarray(wk),
                           np.asarray(wv), np.asarray(wo), np.asarray(pope_bias))
    res = run_bass_kernel_spmd(nc, in_maps, list(range(NCORES)))
    y = np.zeros((DIM, S), np.float32)
    for c in range(NCORES):
        y += res.results[c]["yT"].reshape(DIM, S).astype(np.float32)
    return np.ascontiguousarray(y.T).reshape(1, S, DIM)


if __name__ == "__main__":
    rng = np.random.default_rng(0)
    out = kernel(
        x=rng.standard_normal((1, S, DIM)).astype(np.float32),
        wq=rng.standard_normal((DIM, DIM)).astype(np.float32) / 32,
        wk=rng.standard_normal((DIM, DIM)).astype(np.float32) / 32,
        wv=rng.standard_normal((DIM, DIM)).astype(np.float32) / 32,
        wo=rng.standard_normal((DIM, DIM)).astype(np.float32) / 32,
        pope_bias=-rng.random((H, D), np.float32) * 3.0,
    )
    print("out", out.shape, out.dtype, np.abs(out).mean())


# revision 43
# speedup vs baseline: 1.2676x; 1.0261x over previous
"""PoPE attention kernel for Trainium2, sharded over 8 NeuronCores by heads.

Problem: B=1, S=2048, DIM=1024, H=16 heads, D=64.
  q/k/v = x @ w{q,k,v}^T ; PoPE embed (softplus magnitude x cos/sin phase);
  scores = q_emb @ k_emb^T / sqrt(D); softmax; out = attn @ v; y = out @ wo^T.

Sharding: 2 heads per core. Each core computes its heads' projections,
attention, and a partial output projection (its 128 channels of wo);
host sums the 8 partial y's (f32) - no on-chip collectives.

Layouts are "transposed" (feature-major): xT [DIM, S] so that every
matmul has its contraction on the partition axis with no on-chip
transposes. All matmuls are bf16 with f32 PSUM accumulate.
The softmax skips max-subtraction (scores/8 are bounded ~4.3), and the
rowsum comes free from a ones-column appended to v in the attn@v matmul.
Attention is pipelined in (head, half-of-keys) stages so exp buffers fit
SBUF; attn@v of the previous stage overlaps scores+exp of the current,
and the v-projection fills TensorE gaps during the first stage.
The softmax 1/rowsum uses a DMA reshape to spread the row over 128
partitions (DVE reciprocal is ~6.3ns per element per lane; [1,2048]
would cost 13us, [128,16] costs ~0.1us).
"""
import math

import numpy as np
import ml_dtypes

import concourse.bacc as bacc
import concourse.mybir as mybir
from concourse import tile
from concourse.bass_utils import run_bass_kernel_spmd

BF16 = ml_dtypes.bfloat16
S, DIM, H, D = 2048, 1024, 16, 64
NCORES = 8
HPC = H // NCORES          # heads per core = 2
ED = 2 * D                 # embedding width per head = 128
KI = DIM // 128            # contraction chunks for projections = 8
KC = S // 128              # key-token chunks = 16
QC = S // 512              # query free-dim chunks of 512 = 4
OC = DIM // 128            # output-channel chunks = 8

_compiled_nc = None


def _build_body(nc, tc, persist, ps_pool, out_pool, xt_pool, exp_pool, ext):
    dt = mybir.dt
    AF = mybir.ActivationFunctionType
    xt_ext, w_ext, tq_ext, tk_ext, wo_ext, y_ext = ext
    QH = 1024                  # query superblock width (enables PSUM
    #                            double-buffering of the score tiles)

    # ---- HAM warmup: dummy matmuls on junk data while the input DMAs run,
    # so the PE clock-gate reaches 2.4 GHz before the real matmuls start ----
    warm_sb = persist.tile([128, 512], dt.bfloat16)
    nc.gpsimd.memset(warm_sb[:], 0.0)
    warm_ps = ps_pool.tile([128, 512], dt.float32, name="warm_ps", tag="scA")
    for i in range(16):
        nc.tensor.matmul(warm_ps[:], warm_sb[:, 0:128], warm_sb[:],
                         start=(i == 0), stop=(i == 15))

    # ---- phase A: input DMAs (w and xt first so matmuls start early) ----
    w_sb = persist.tile([128, 3, KI, ED], dt.bfloat16)
    nc.sync.dma_start(w_sb[:], w_ext[:])
    xt = xt_pool.tile([128, KI, S], dt.bfloat16)
    for ki in range(KI):
        nc.sync.dma_start(xt[:, ki, :], xt_ext[:, ki, :])
    tq_sb = persist.tile([128, 2, S], dt.bfloat16)
    nc.sync.dma_start(tq_sb[:], tq_ext[:])
    tk_sb = persist.tile([128, 2, S], dt.bfloat16)
    nc.sync.dma_start(tk_sb[:], tk_ext[:])
    wo_sb = persist.tile([128, DIM], dt.bfloat16)
    nc.sync.dma_start(wo_sb[:], wo_ext[:])
    # v with a ones column appended per (head, key chunk)
    v_sb = persist.tile([128, HPC, KC, D + 1], dt.bfloat16)
    nc.gpsimd.memset(v_sb[:, 0, :, D], 1.0)
    nc.gpsimd.memset(v_sb[:, 1, :, D], 1.0)
    emb_q = [persist.tile([128, S], dt.bfloat16, name=f"embq{h}", tag=f"embq{h}")
             for h in range(HPC)]
    emb_k = [persist.tile([128, S], dt.bfloat16, name=f"embk{h}", tag=f"embk{h}")
             for h in range(HPC)]
    outT = persist.tile([128, S], dt.bfloat16)

    # PSUM layout: four 2-bank tags. Scores ping-pong on scA/scB while the
    # two attn@v accumulators sit on avA/avB; the projections and the output
    # projection reuse the same four tags.
    # ---- phase B: q/k projections (ki-outer so MMs start after the first
    # xt chunk lands), clustered softplus ----
    psm = {}
    for p in range(2):  # 0=q, 1=k
        for lo in range(2):
            t = ps_pool.tile([128, QH], dt.float32, name=f"psm{p}_{lo}",
                             tag=("scA", "avA", "scB", "avB")[2 * p + lo])
            psm[(p, lo)] = t
        for ki in range(KI):
            for qc in range(QC):
                nc.tensor.matmul(
                    psm[(p, qc // 2)][:, (qc % 2) * 512:(qc % 2) * 512 + 512],
                    w_sb[:, p, ki, :],
                    xt[:, ki, qc * 512:(qc + 1) * 512],
                    start=(ki == 0), stop=(ki == KI - 1),
                )
    # softplus(x) = ln(1 + e^x); all four Exp quarters feed ONE combined Ln
    # so the scheduler cannot interleave exp/ln (different ACT table sets;
    # each switch costs a ~1.3us table load).
    tmp = xt_pool.tile([128, 2, S], dt.float32, name="sp", tag="sp")
    mag = xt_pool.tile([128, 2, S], dt.bfloat16, name="mag", tag="mag")
    qk_mag = [mag[:, 0, :], mag[:, 1, :]]
    for p in range(2):
        for lo in range(2):
            nc.scalar.activation(tmp[:, p, lo * QH:(lo + 1) * QH],
                                 psm[(p, lo)][:], AF.Exp)
    nc.scalar.activation(mag[:, :, 0:QH], tmp[:, :, 0:QH], AF.Ln, bias=1.0)
    nc.scalar.activation(mag[:, :, QH:S], tmp[:, :, QH:S], AF.Ln, bias=1.0)

    # ---- phase C: v projection (token-major directly) ----
    def v_group(g, tag):
        psv = ps_pool.tile([128, 4, 128], dt.float32, name=f"psv{g}", tag=tag)
        for sub in range(4):
            t = 4 * g + sub
            for ki in range(KI):
                nc.tensor.matmul(
                    psv[:, sub, :],
                    xt[:, ki, t * 128:(t + 1) * 128],
                    w_sb[:, 2, ki, :],
                    start=(ki == 0), stop=(ki == KI - 1),
                )
        for h in range(HPC):
            for sub in range(4):
                t = 4 * g + sub
                nc.vector.tensor_copy(
                    v_sb[:, h, t, 0:D], psv[:, sub, 64 * h:64 * h + 64])

    v_group(0, "scA")
    v_group(1, "scB")

    # embeds on DVE (bf16 SBUF 2x mode), in column halves with head 0's
    # low half first so its first score chunks can start sooner
    for lo in range(2):
        c = slice(lo * QH, (lo + 1) * QH)
        for h in range(HPC):
            r = slice(64 * h, 64 * h + 64)
            for t in range(2):  # 0=cos part, 1=sin part
                e = slice(64 * t, 64 * t + 64)
                nc.vector.tensor_mul(emb_q[h][e, c], qk_mag[0][r, c], tq_sb[r, t, c])
                nc.vector.tensor_mul(emb_k[h][e, c], qk_mag[1][r, c], tk_sb[r, t, c])

    # ---- phase D: attention in two query superblocks of 1024; inside each,
    # (head, key-half) stages. Score tiles double-buffer on scA/scB so the
    # next chunk's matmuls overlap the previous chunk's exp eviction. ----
    av_ps = {}
    exp_tiles = {}

    def scores_chunk(h, kc, qh):
        e = exp_pool.tile([128, QH], dt.bfloat16,
                          name=f"exp{qh}_{h}_{kc}", tag=f"exp{qh}_{kc % 8}")
        exp_tiles[(h, kc, qh)] = e
        sc = ps_pool.tile([128, QH], dt.float32, name=f"sc{qh}_{h}_{kc}",
                          tag=("scA", "scB")[kc % 2])
        for q2 in range(2):
            nc.tensor.matmul(
                sc[:, q2 * 512:(q2 + 1) * 512],
                emb_k[h][:, kc * 128:(kc + 1) * 128],
                emb_q[h][:, qh * QH + q2 * 512:qh * QH + (q2 + 1) * 512],
                start=True, stop=True,
            )
        nc.scalar.activation(e[:], sc[:], AF.Exp, scale=1.0 / math.sqrt(D))

    def av_chunk(h, kc, qh):
        pav = av_ps[(h, qh)]
        e = exp_tiles[(h, kc, qh)]
        for q2 in range(2):
            nc.tensor.matmul(
                pav[0:D + 1, q2 * 512:(q2 + 1) * 512],
                v_sb[:, h, kc, :],
                e[:, q2 * 512:(q2 + 1) * 512],
                start=(kc == 0), stop=(kc == KC - 1),
            )

    def normalize(h, qh):
        # Evict the [65, QH] attn@v accumulator to SBUF immediately so the
        # PSUM slot frees fast; then 1/rowsum via DMA spread across 128
        # partitions (DVE reciprocal is free-size bound ~6.3 ns/elem/lane),
        # gpsimd broadcast, multiply.
        pav = av_ps[(h, qh)]
        acopy = persist.tile([D + 1, QH], dt.float32,
                             name=f"acopy{h}_{qh}", tag=f"acopy{h}")
        if h == 0 or qh == 0:
            # ACT is mid-exp-stream: evict on DVE
            nc.vector.tensor_copy(acopy[:], pav[0:D + 1, :])
        else:
            nc.scalar.activation(acopy[:], pav[0:D + 1, :], AF.Copy)
        rs128 = persist.tile([128, QH // 128], dt.float32,
                             name=f"rs128_{h}_{qh}", tag="rs128")
        nc.sync.dma_start(rs128[:], acopy[D:D + 1, :])
        rr128 = persist.tile([128, QH // 128], dt.bfloat16,
                             name=f"rr128_{h}_{qh}", tag="rr128")
        with nc.allow_low_precision(reason="softmax 1/rowsum in bf16 is ~0.4% scale noise"):
            nc.vector.reciprocal(rr128[:], rs128[:])
        rr = persist.tile([1, QH], dt.bfloat16, name=f"rr{h}_{qh}", tag="rr")
        nc.sync.dma_start(rr[:], rr128[:])
        rsb = persist.tile([64, QH], dt.bfloat16, name=f"rsb{h}_{qh}", tag="rsb")
        nc.gpsimd.partition_broadcast(rsb[:], rr[:])
        nc.vector.tensor_mul(outT[64 * h:64 * h + 64, qh * QH:(qh + 1) * QH],
                             acopy[0:D, :], rsb[:])

    def oproj(qh, oc, tags=("avA", "avB"), act_ok=False):
        c = slice(qh * QH, (qh + 1) * QH)
        psy = ps_pool.tile([128, QH], dt.float32, name=f"psy{qh}_{oc}",
                           tag=tags[oc % len(tags)])
        for q2 in range(2):
            nc.tensor.matmul(
                psy[:, q2 * 512:(q2 + 1) * 512],
                wo_sb[:, oc * 128:(oc + 1) * 128],
                outT[:, qh * QH + q2 * 512:qh * QH + (q2 + 1) * 512],
                start=True, stop=True,
            )
        y_sb = out_pool.tile([128, QH], dt.bfloat16, name=f"y{qh}_{oc}", tag="y")
        if act_ok:
            # ACT is idle here: split the eviction across ACT and DVE halves
            nc.scalar.activation(y_sb[:, 0:QH // 2], psy[:, 0:QH // 2], AF.Copy)
            nc.vector.tensor_copy(y_sb[:, QH // 2:QH], psy[:, QH // 2:QH])
        else:
            # ACT is saturated with exps: keep the eviction off it
            nc.vector.tensor_copy(y_sb[:], psy[:])
        nc.sync.dma_start(y_ext[oc, :, c], y_sb[:])

    def oproj_h0(qh, oc, tag):
        # first half of the contraction (head 0's 64 channels); runs before
        # the last normalize, filling the PE idle window
        psy = ps_pool.tile([128, QH], dt.float32, name=f"psyS{qh}_{oc}", tag=tag)
        for q2 in range(2):
            nc.tensor.matmul(
                psy[:, q2 * 512:(q2 + 1) * 512],
                wo_sb[0:64, oc * 128:(oc + 1) * 128],
                outT[0:64, qh * QH + q2 * 512:qh * QH + (q2 + 1) * 512],
                start=True, stop=False,
            )
        return psy

    def oproj_h1(qh, oc, psy):
        c = slice(qh * QH, (qh + 1) * QH)
        for q2 in range(2):
            nc.tensor.matmul(
                psy[:, q2 * 512:(q2 + 1) * 512],
                wo_sb[64:128, oc * 128:(oc + 1) * 128],
                outT[64:128, qh * QH + q2 * 512:qh * QH + (q2 + 1) * 512],
                start=False, stop=True,
            )
        y_sb = out_pool.tile([128, QH], dt.bfloat16, name=f"yS{qh}_{oc}", tag="y")
        nc.scalar.activation(y_sb[:, 0:QH // 2], psy[:, 0:QH // 2], AF.Copy)
        nc.vector.tensor_copy(y_sb[:, QH // 2:QH], psy[:, QH // 2:QH])
        nc.sync.dma_start(y_ext[oc, :, c], y_sb[:])

    LAG = 1
    for qh in range(2):
        # stage 0: scores/exp (h0, 1st key half); fillers keep PE dense:
        # superblock 0 runs the last v-projection groups, superblock 1 runs
        # superblock 0's output projection on the freed avA/avB banks.
        for j in range(8):
            if qh == 0:
                if j == 0:
                    v_group(2, "avA")
                if j == 3:
                    v_group(3, "avB")
            if qh == 1:
                oproj(0, j)
            scores_chunk(0, j, qh)
        # stage 1: scores/exp (h0, 2nd half) + av (h0, 1st half)
        av_ps[(0, qh)] = ps_pool.tile([128, QH], dt.float32,
                                      name=f"av0_{qh}", tag="avA")
        for j in range(8):
            av_chunk(0, j, qh)
            scores_chunk(0, 8 + j, qh)
        # stage 2: scores/exp (h1, 1st half) + av (h0, 2nd half)
        av_ps[(1, qh)] = ps_pool.tile([128, QH], dt.float32,
                                      name=f"av1_{qh}", tag="avB")
        for j in range(8):
            av_chunk(0, 8 + j, qh)
            scores_chunk(1, j, qh)
        normalize(0, qh)
        # stage 3: scores/exp (h1, 2nd half) + av (h1, 1st half)
        for j in range(8):
            if j >= LAG:
                av_chunk(1, j - LAG, qh)
            scores_chunk(1, 8 + j, qh)
        for kc in range(8 - LAG, KC):
            av_chunk(1, kc, qh)
        normalize(1, qh)

    # ---- phase E: output projection for the last superblock (scA/scB are
    # free after the last exp -> 4-deep psy pipeline) ----
    for oc in range(OC):
        oproj(1, oc, tags=("avA", "avB", "scA", "scB"), act_ok=True)


def _build():
    nc = bacc.Bacc()
    dt = mybir.dt

    ext = (
        nc.declare_dram_parameter("xt", [128, KI, S], dt.bfloat16, isOutput=False),
        nc.declare_dram_parameter("w", [128, 3, KI, ED], dt.bfloat16, isOutput=False),
        nc.declare_dram_parameter("trig_q", [128, 2, S], dt.bfloat16, isOutput=False),
        nc.declare_dram_parameter("trig_k", [128, 2, S], dt.bfloat16, isOutput=False),
        nc.declare_dram_parameter("woT", [128, DIM], dt.bfloat16, isOutput=False),
        nc.declare_dram_parameter("yT", [OC, 128, S], dt.bfloat16, isOutput=True),
    )

    with tile.TileContext(nc) as tc:
        with tc.tile_pool(name="persist", bufs=1) as persist, \
             tc.tile_pool(name="ps", bufs=1, space="PSUM") as ps_pool, \
             tc.tile_pool(name="out", bufs=2) as out_pool, \
             tc.tile_pool(name="xtp", bufs=1) as xt_pool, \
             tc.tile_pool(name="expp", bufs=2) as exp_pool:
            _build_body(nc, tc, persist, ps_pool, out_pool, xt_pool, exp_pool, ext)
    nc.compile()
    return nc


def _get_nc():
    global _compiled_nc
    if _compiled_nc is None:
        _compiled_nc = _build()
    return _compiled_nc


def _prep_inputs(x, wq, wk, wv, wo, pope_bias):
    """Host-side sharding + layout prep. Returns in_maps for the 8 cores."""
    x2 = np.ascontiguousarray(x.reshape(S, DIM).astype(np.float32))

    # trig tables (f64 phases for accuracy)
    inv = 10000.0 ** (-(np.arange(D, dtype=np.float64) / D))
    pos = np.arange(S, dtype=np.float64)
    freqs = pos[:, None] * inv[None, :]                       # [S, D]
    bias = np.clip(pope_bias.astype(np.float64), -2 * math.pi, 0.0)  # [H, D]

    cos_q = np.cos(freqs).T.astype(BF16)                      # [D, S]
    sin_q = np.sin(freqs).T.astype(BF16)
    trig_q = np.empty((128, 2, S), BF16)
    trig_q[0:64, 0] = cos_q
    trig_q[64:128, 0] = cos_q
    trig_q[0:64, 1] = sin_q
    trig_q[64:128, 1] = sin_q

    # xt[q, ki, s] = x[s, ki*128+q]
    xt = np.ascontiguousarray(
        x2.T.reshape(KI, 128, S).transpose(1, 0, 2)).astype(BF16)

    in_maps = []
    for c in range(NCORES):
        hs = slice(c * HPC * D, (c + 1) * HPC * D)            # head-channel slice
        # lhsT chunk for proj p is w_p[hs].T[ki*128:(ki+1)*128, :]
        w = np.empty((128, 3, KI, ED), BF16)
        for p, wm in enumerate((wq, wk, wv)):
            wt = np.ascontiguousarray(wm[hs, :].astype(np.float32).T)  # [DIM, ED]
            w[:, p] = wt.reshape(KI, 128, ED).transpose(1, 0, 2)

        ph = freqs[None, :, :] + bias[c * HPC:(c + 1) * HPC, None, :]  # [HPC, S, D]
        trig_k = np.empty((128, 2, S), BF16)
        for h in range(HPC):
            trig_k[64 * h:64 * h + 64, 0] = np.cos(ph[h]).T
            trig_k[64 * h:64 * h + 64, 1] = np.sin(ph[h]).T

        woT = np.ascontiguousarray(wo[:, hs].astype(np.float32).T).astype(BF16)

        in_maps.append({
            "xt": xt, "w": w, "trig_q": trig_q, "trig_k": trig_k, "woT": woT,
        })
    return in_maps


def kernel(x, wq, wk, wv, wo, pope_bias):
    nc = _get_nc()
    in_maps = _prep_inputs(np.asarray(x), np.asarray(wq), np.asarray(wk),
                           np.asarray(wv), np.asarray(wo), np.asarray(pope_bias))
    res = run_bass_kernel_spmd(nc, in_maps, list(range(NCORES)))
    y = np.zeros((DIM, S), np.float32)
    for c in range(NCORES):
        y += res.results[c]["yT"].reshape(DIM, S).astype(np.float32)
    return np.ascontiguousarray(y.T).reshape(1, S, DIM)


if __name__ == "__main__":
    rng = np.random.default_rng(0)
    out = kernel(
        x=rng.standard_normal((1, S, DIM)).astype(np.float32),
        wq=rng.standard_normal((DIM, DIM)).astype(np.float32) / 32,
        wk=rng.standard_normal((DIM, DIM)).astype(np.float32) / 32,
        wv=rng.standard_normal((DIM, DIM)).astype(np.float32) / 32,
        wo=rng.standard_normal((DIM, DIM)).astype(np.float32) / 32,
        pope_bias=-rng.random((H, D), np.float32) * 3.0,
    )
    print("out", out.shape, out.dtype, np.abs(out).mean())
